# revision 42
# baseline (speedup 1.0000x reference)
"""Trainium2 Bass kernel for a GPT-style causal attention block.

  y = proj( softmax_causal( (x@Wq)(x@Wk)^T / sqrt(hd) ) @ (x@Wv) )

Shapes (hardcoded): B=2, S=2048, D=1024, H=16 heads, hd=64.

Sharding over 8 NeuronCores: core = (batch b, head-group g), g selects 4
heads (2 head PAIRS). Each core:

  phase 1+2 (fused): QKV projection chains are PREFETCHED into the
    attention matmul stream one query-block ahead (the PE never drains at
    block seams, keeping its DVFS p-state at the 2.4 GHz ceiling). q,k
    produced TRANSPOSED [head_ch, S]; k gets NO bias add (a key bias
    shifts every logit of a softmax row equally - provably cancels), the
    q bias is one DVE scalar-add per block. v is natural [S, head_ch]
    padded with 64 ONES columns: the AV matmul then replicates the
    softmax denominator across psum rows 64..127 for free. Scores run as
    head PAIRS in the transposed layout [key, query] with the two K=64
    matmuls CONCURRENT in the PE via row-group tile_position; exp on ACT
    (scale folded); causal multiply only on the [128,128] diagonal
    triangle (DVE). AV(jt) is emitted after scores(jt+1) so exp latency
    never head-of-line blocks the PE. Normalize (reciprocal * av) is
    emitted mid-stream right after the diagonal masks of the NEXT query
    block, one block deferred.

  phase 3 (distributed): SIX fine-grained AllGathers, each triggered
    mid-stream the moment its aT span is normalized (pair 0 in halves,
    pair 1 per query block), so each collective's ~10-40us ncfw flight
    hides under the remaining attention stream instead of serializing at
    the end (remote_dma SBUF->SBUF peer writes were tried and are
    architecturally slow here: one descriptor per partition = ~75us per
    [128,512] chunk). The output projection consumes gathered rows per
    s-quarter: sq0 interleaved late into the last attention block, sq1-3
    in the tail where sq1/sq2 (data long arrived) cover the final
    collective's flight before sq3 needs it.

DMA order is critical-path aware: the q/k weight slices pair 0 needs and
the first quarter of xT are issued first, the rest streams under compute.
"""

import os as _os

# collectives leave mesh state behind; a stale mesh in a fresh process
# fails with "mesh desynced" unless the cores reset at runtime init
_os.environ.setdefault("NEURON_RT_RESET_CORES", "1")

import numpy as np

B = 2
S = 2048
D = 1024
H = 16
HD = 64
HLOC = 4          # heads per core
NPAIR = 2         # head pairs per core
N_CORES = 8
GROUP = 4         # cores per batch (replica group size)
IB = 512          # query block width (matmul moving dim)
OC = D // GROUP   # output-projection column shard per core (256)
SCALE = 1.0 / 8.0  # 1/sqrt(hd)


def _build_bass(s=S):
    """Build the SPMD Bass program (one NeuronCore's view)."""
    import concourse.bacc as bacc
    import concourse.mybir as mybir
    import concourse.tile as tile

    f32 = mybir.dt.float32
    f32r = mybir.dt.float32r
    bf16 = mybir.dt.bfloat16
    Alu = mybir.AluOpType
    Act = mybir.ActivationFunctionType

    n_ib = s // IB           # query blocks (4)
    n_st = s // 128          # 128-row sequence tiles (16)
    n_dt = D // 128          # contraction tiles for D (8)

    nc = bacc.Bacc(num_devices=N_CORES)

    xt = nc.declare_dram_parameter("xt", [D, s], bf16, isOutput=False)
    wqk = nc.declare_dram_parameter("wqk", [D, 512], bf16, isOutput=False)
    wv = nc.declare_dram_parameter("wv", [D, 256], bf16, isOutput=False)
    bq = nc.declare_dram_parameter("bq", [128, NPAIR], f32, isOutput=False)
    bv = nc.declare_dram_parameter("bv", [128, 256], f32, isOutput=False)
    wp = nc.declare_dram_parameter("wp", [D, OC], bf16, isOutput=False)
    bp = nc.declare_dram_parameter("bp", [128, 2], f32, isOutput=False)
    msk = nc.declare_dram_parameter("msk", [128, 128], f32, isOutput=False)
    # y TRANSPOSED [oc, s]: the projection runs as long psum chains
    y = nc.declare_dram_parameter("y", [OC, s], f32, isOutput=True)
    import os as _os
    _dbg = bool(_os.environ.get("KERNEL_DEBUG_DUMP"))
    if _dbg:
        adbg = nc.declare_dram_parameter(
            "adbg", [128, NPAIR * s], bf16, isOutput=True
        )

    with tile.TileContext(nc) as tc:
        with (
            tc.tile_pool(name="const", bufs=1) as const,
            tc.tile_pool(name="persist", bufs=1) as persist,
            tc.tile_pool(name="dram", bufs=1, space="DRAM") as dram,
        ):
            # ---- allocations (emit nothing) ----
            bq_sb = const.tile([128, NPAIR], f32)
            bv_sb = const.tile([128, 256], f32)
            bp_sb = const.tile([128, 2], f32)
            msk_sb = const.tile([128, 128], f32)
            wp_sb = const.tile([128, n_dt, OC], bf16)
            warm_sb = const.tile([1, 1], f32)
            ones_sb = const.tile([128, 256], f32)

            xt_sb = persist.tile([128, n_dt, s], bf16)
            wqk_sb = persist.tile([128, n_dt, 512], bf16)
            wv_sb = persist.tile([128, n_dt, 256], bf16)
            # bf16 score/AV operands: halves LDWEIGHTS traffic (the AV
            # lhsT is reloaded per key tile) and dodges the f32r
            # small-moving-dim matmul penalty on diagonal tiles
            qT_sb = persist.tile([128, NPAIR, s], bf16)   # [pair_ch, pair, s]
            kT_sb = persist.tile([128, NPAIR, s], bf16)
            # [:, st, h, 0:64] = v channels, [:, st, h, 64:128] = 1.0
            v_sb = persist.tile([128, n_st, HLOC, 128], bf16)
            aT_sb = persist.tile([128, NPAIR, s], bf16)
            # gathered rows: [:, pair, rank, :] = rank's pair rows
            agf_sb = persist.tile([128, NPAIR, GROUP, s], bf16)

            # fine-grained collective spans: (pair, col_lo, col_hi),
            # triggered right after normalize of (key_pair, key_ib)
            AGS = {
                (0, 0): (0, 0, IB),              # pair0 per block: front-
                (0, 1): (0, IB, s // 2),         # load the CC pipeline
                (0, 2): (0, s // 2, 3 * IB),
                (0, 3): (0, 3 * IB, s),
                (1, 1): (1, 0, s // 2),          # pair1 first half
                (1, 2): (1, s // 2, 3 * IB),     # pair1 third quarter
                (1, 3): (1, 3 * IB, s),          # pair1 last block (small:
            }                                    # its flight is the tail)
            ag_in = {
                k: dram.tile([128, hi - lo], bf16, name=f"agi{k[0]}{k[1]}")
                for k, (p, lo, hi) in AGS.items()
            }
            ag_out = {
                k: dram.tile(
                    [GROUP * 128, hi - lo], bf16, name=f"ago{k[0]}{k[1]}"
                )
                for k, (p, lo, hi) in AGS.items()
            }

            # ---- DMA emission, critical-path order ----
            # 1. exactly what pair-0 block-0 needs: q/k pair-0 weight
            #    slices + first xT quarter.
            wqk_r = wqk.rearrange("(t p) c -> p t c", p=128)
            xt_r = xt.rearrange("(t p) ss -> p t ss", p=128)
            nc.sync.dma_start(out=wqk_sb[:, :, 0:128], in_=wqk_r[:, :, 0:128])
            # first xT quarter in dt-halves: the qk chain's dt-0..3
            # matmuls start before the 4..7 tiles land
            nc.sync.dma_start(
                out=xt_sb[:, 0:4, 0 : s // 4], in_=xt_r[:, 0:4, 0 : s // 4]
            )
            nc.sync.dma_start(
                out=wqk_sb[:, :, 256:384], in_=wqk_r[:, :, 256:384]
            )
            nc.sync.dma_start(
                out=xt_sb[:, 4:8, 0 : s // 4], in_=xt_r[:, 4:8, 0 : s // 4]
            )
            nc.sync.dma_start(out=bq_sb, in_=bq[:, :])
            nc.sync.dma_start(
                out=wv_sb, in_=wv.rearrange("(t p) c -> p t c", p=128)
            )
            nc.sync.dma_start(out=msk_sb, in_=msk[:, :])
            nc.sync.dma_start(out=bv_sb, in_=bv[:, :])
            # dummy exp: pulls the ACT exp table load off the critical path
            nc.scalar.activation(
                out=warm_sb, in_=bq_sb[0:1, 0:1], func=Act.Exp, scale=0.0
            )
            nc.vector.memset(ones_sb, 1.0)
            # 2. rest streams under compute
            nc.sync.dma_start(
                out=wqk_sb[:, :, 128:256], in_=wqk_r[:, :, 128:256]
            )
            nc.sync.dma_start(
                out=wqk_sb[:, :, 384:512], in_=wqk_r[:, :, 384:512]
            )
            for q in range(1, 4):
                nc.sync.dma_start(
                    out=xt_sb[:, :, q * s // 4 : (q + 1) * s // 4],
                    in_=xt_r[:, :, q * s // 4 : (q + 1) * s // 4],
                )
            nc.sync.dma_start(out=bp_sb, in_=bp[:, :])
            nc.sync.dma_start(
                out=wp_sb, in_=wp.rearrange("(t p) c -> p t c", p=128)
            )

            with (
                tc.tile_pool(name="ps_s", bufs=2, space="PSUM") as ps_s,
                tc.tile_pool(name="ps_av", bufs=2, space="PSUM") as ps_av,
                tc.tile_pool(name="pt", bufs=5) as ptpool,
                tc.tile_pool(name="small", bufs=6) as small,
                tc.tile_pool(name="yout", bufs=3) as yout,
            ):
                # ---------- chain emitters (PE work units) ----------
                def v_chain(st):
                    # v natural: lhsT = xT tile [d, s-tile], rhs = Wv
                    psv = ps_s.tile([128, 256], f32, name="psv", tag="pss")
                    for dt in range(n_dt):
                        nc.tensor.matmul(
                            psv,
                            lhsT=(xt_sb[:, dt, st * 128 : (st + 1) * 128]),
                            rhs=(wv_sb[:, dt, :]),
                            start=(dt == 0),
                            stop=(dt == n_dt - 1),
                        )
                    nc.vector.tensor_tensor(
                        out=v_sb[:, st, :, 0:64],
                        in0=psv.rearrange("p (h e) -> p h e", h=HLOC),
                        in1=bv_sb.rearrange("p (h e) -> p h e", h=HLOC),
                        op=Alu.add,
                    )
                    # ones columns: AV replicates the softmax denominator
                    # across psum rows 64..127
                    nc.vector.tensor_copy(
                        out=v_sb[:, st, :, 64:128],
                        in_=ones_sb.rearrange("p (h e) -> p h e", h=HLOC),
                    )

                def qk_chain(t, sb):
                    # qT/kT: lhsT = W tile [d,c], rhs = xT [d, s-block]
                    # c-tile t: 0,1 = q pair0/1; 2,3 = k pair0/1
                    ps = ps_s.tile([128, IB], f32, name="ps", tag="pss")
                    for dt in range(n_dt):
                        nc.tensor.matmul(
                            ps,
                            lhsT=(wqk_sb[:, dt, t * 128 : (t + 1) * 128]),
                            rhs=(xt_sb[:, dt, sb * IB : (sb + 1) * IB]),
                            start=(dt == 0),
                            stop=(dt == n_dt - 1),
                        )
                    if t < 2:  # q: bias add (k bias cancels in softmax)
                        nc.vector.tensor_scalar_add(
                            out=qT_sb[:, t, sb * IB : (sb + 1) * IB],
                            in0=ps,
                            scalar1=bq_sb[:, t : t + 1],
                        )
                    else:
                        nc.vector.tensor_copy(
                            out=kT_sb[:, t - 2, sb * IB : (sb + 1) * IB],
                            in_=ps,
                        )

                def proj_chain(sq, ocb):
                    """yT[ocb rows, s-quarter sq] = sum_t wp_t^T @ a_t."""
                    lo, hi = sq * s // 4, (sq + 1) * s // 4
                    yt = ps_s.tile([128, s // 4], f32, name="yt", tag="pss")
                    t = 0
                    for pair in range(NPAIR):
                        for r in range(GROUP):
                            nc.tensor.matmul(
                                yt,
                                lhsT=(wp_sb[:, t, ocb * 128 : (ocb + 1) * 128]),
                                rhs=agf_sb[:, pair, r, lo:hi],
                                start=(t == 0),
                                stop=(t == 2 * GROUP - 1),
                            )
                            t += 1
                    ysb = yout.tile([128, s // 4], f32, name="ysb")
                    nc.vector.tensor_scalar_add(
                        out=ysb, in0=yt, scalar1=bp_sb[:, ocb : ocb + 1]
                    )
                    nc.sync.dma_start(
                        out=y[ocb * 128 : (ocb + 1) * 128, lo:hi], in_=ysb
                    )

                def proj_emit(sq):
                    proj_chain(sq, 0)
                    proj_chain(sq, 1)

                # ---------- softmax-side emitters ----------
                def normalize_emit(pair, ib, avs):
                    """aT[:, ib block] = av[0:64] * (1 / av[64:128])."""
                    for hh in range(2):
                        zsb = small.tile([64, IB], f32, name="zsb")
                        nc.vector.tensor_copy(out=zsb, in_=avs[hh][64:128, :])
                        recr = small.tile([64, IB], f32, name="recr")
                        nc.vector.reciprocal_approx_fast(out=recr, in_=zsb)
                        nc.vector.tensor_tensor(
                            out=aT_sb[
                                hh * 64 : (hh + 1) * 64,
                                pair,
                                ib * IB : (ib + 1) * IB,
                            ],
                            in0=recr,
                            in1=avs[hh][0:64, :],
                            op=Alu.mult,
                        )

                def send_emit(pair, ib):
                    """Stage + AllGather + reload the collective span
                    keyed by (pair, ib), if any."""
                    key = (pair, ib)
                    if key not in AGS:
                        return
                    p, lo, hi = AGS[key]
                    gin, gout = ag_in[key], ag_out[key]
                    nc.sync.dma_start(out=gin, in_=aT_sb[:, p, lo:hi])
                    nc.gpsimd.collective_compute(
                        "AllGather",
                        Alu.bypass,
                        replica_groups=[[0, 2, 4, 6], [1, 3, 5, 7]],
                        ins=[gin[:, :]],
                        outs=[gout[:, :]],
                    )
                    nc.sync.dma_start(
                        out=agf_sb[:, p, :, lo:hi],
                        in_=gout.rearrange("(r p) ss -> p r ss", p=128),
                    )

                # ---------- the fused attention stream ----------
                def scores_av_emit(pair, ib, fillers, pending):
                    """Scores + exp + causal mask + AV for query block ib.
                    `fillers` are chain thunks interleaved into the jt
                    stream; `pending` (avs of the previous block) is
                    normalized + shipped right after the diagonal masks.
                    Returns the two [128,IB] psum accumulators (rows
                    64..127 = replicated softmax denominator)."""
                    njt = n_ib * (ib + 1)  # key tiles needed (j <= i)
                    avs = [
                        ps_av.tile([128, IB], f32, name=f"av{hh}", tag=f"av{hh}")
                        for hh in range(2)
                    ]
                    # diagonal key tiles first: their mask multiply then
                    # overlaps the long non-diagonal score/AV stream
                    jt_order = list(range(n_ib * ib, njt)) + list(
                        range(n_ib * ib)
                    )
                    fill = list(fillers)
                    # spread fillers over the post-diagonal jt slots
                    slots = {}
                    nseg = len(fill) + 1
                    for m in range(len(fill)):
                        pos = 4 + (m + 1) * max(njt - 4, 1) // nseg
                        slots.setdefault(min(pos, njt - 1), []).append(fill[m])

                    def av_emit(pt, jt, lo, jseq):
                        for hh in range(2):
                            nc.tensor.matmul(
                                avs[hh][:, lo:IB],
                                lhsT=(v_sb[:, jt, pair * 2 + hh, :]),
                                rhs=(pt[:, hh, lo:IB]),
                                start=(jseq == 0),
                                stop=(jseq == njt - 1),
                            )

                    prev_av = None
                    for jseq, jt in enumerate(jt_order):
                        k = jt - n_ib * ib  # >= 0: diagonal tile index
                        lo = 128 * k if k > 0 else 0
                        pss = ps_s.tile(
                            [128, 2 * IB], f32, name="pss", tag="pss"
                        )
                        for hh in range(2):
                            off = hh * 64
                            nc.tensor.matmul(
                                pss[:, hh * IB + lo : (hh + 1) * IB],
                                lhsT=(kT_sb[
                                        off : off + 64,
                                        pair,
                                        jt * 128 : (jt + 1) * 128,
                                    ]
                                ),
                                rhs=(qT_sb[
                                        off : off + 64,
                                        pair,
                                        ib * IB + lo : (ib + 1) * IB,
                                    ]
                                ),
                                start=True,
                                stop=True,
                                tile_position=(off, 0),
                            )
                        if prev_av is not None:
                            av_emit(*prev_av)
                        pt = ptpool.tile([128, 2, IB], bf16, name="pt")
                        nc.scalar.activation(
                            out=pt[:, :, lo:IB],
                            in_=pss.rearrange("p (h q) -> p h q", h=2)[
                                :, :, lo:IB
                            ],
                            func=Act.Exp,
                            scale=SCALE,
                        )
                        for hh in range(2):
                            if k >= 0:  # causal triangle on diagonal block
                                nc.vector.tensor_tensor(
                                    out=pt[:, hh, lo : lo + 128],
                                    in0=pt[:, hh, lo : lo + 128],
                                    in1=msk_sb,
                                    op=Alu.mult,
                                )
                        prev_av = (pt, jt, lo, jseq)
                        # after the diagonals: previous block's normalize
                        # + collective ship, then any prefetch fillers
                        if jseq == min(3, njt - 1) and pending is not None:
                            p_pair, p_ib, p_avs = pending
                            normalize_emit(p_pair, p_ib, p_avs)
                            send_emit(p_pair, p_ib)
                            pending = None
                        for th in slots.get(jseq, ()):
                            th()
                    av_emit(*prev_av)
                    return avs

                # ---------- main schedule ----------
                # pipeline fill: q/k chains FIRST (their weights are the
                # first DMAs to land; v chains need wv which arrives
                # later, and the PE queue is in-order)
                qk_chain(0, 0)
                qk_chain(2, 0)
                for st in range(4):
                    v_chain(st)

                seq = [(p, ib) for p in range(NPAIR) for ib in range(n_ib)]
                pending = None
                for i, (pair, ib) in enumerate(seq):
                    nxt = seq[i + 1] if i + 1 < len(seq) else None
                    fillers = []
                    if pair == 0 and ib < n_ib - 1:
                        # next block's v tiles
                        fillers += [
                            (lambda st=st: v_chain(st))
                            for st in range(4 * (ib + 1), 4 * (ib + 2))
                        ]
                    if nxt is not None:
                        # prefetch the next stream's q/k blocks
                        fillers += [
                            lambda t=nxt[0], sb=nxt[1]: qk_chain(t, sb),
                            lambda t=2 + nxt[0], sb=nxt[1]: qk_chain(t, sb),
                        ]
                    if (pair, ib) == (1, 3):
                        # sq0+sq1 in the last block's stream (both need
                        # only pair0 quarters + pair1 first half, all
                        # gathered long ago); single chains per slot so
                        # both psum slots are never held at once
                        fillers += [
                            lambda: proj_chain(0, 0),
                            lambda: proj_chain(0, 1),
                            lambda: proj_chain(1, 0),
                            lambda: proj_chain(1, 1),
                        ]
                    avs = scores_av_emit(pair, ib, fillers, pending)
                    pending = (pair, ib, avs)
                # tail: last chunk's normalize + collective; quarter sq2
                # (data long arrived) covers its flight before sq3
                p_pair, p_ib, p_avs = pending
                normalize_emit(p_pair, p_ib, p_avs)
                send_emit(p_pair, p_ib)
                proj_emit(2)
                proj_emit(3)
                if _dbg:
                    nc.sync.dma_start(
                        out=adbg[:, :], in_=aT_sb.rearrange("p a b -> p (a b)")
                    )

    nc.compile()
    return nc


def _shard_inputs(x, w_attn, b_attn, w_proj, b_proj, s=S):
    """Host-side sharding: build the per-core input maps."""
    import ml_dtypes

    bfl = ml_dtypes.bfloat16
    x = np.asarray(x, dtype=np.float32)
    w_attn = np.asarray(w_attn, dtype=np.float32)
    b_attn = np.asarray(b_attn, dtype=np.float32)
    w_proj = np.asarray(w_proj, dtype=np.float32)
    b_proj = np.asarray(b_proj, dtype=np.float32)

    # causal triangle tile: msk[j, i] = 1.0 if i >= j
    msk = (np.arange(128)[None, :] >= np.arange(128)[:, None]).astype(
        np.float32
    )

    # w_proj rows in gather-rank order: t = (pair, rank): rows of
    # head-group `rank`'s heads (2*pair, 2*pair+1)
    perm = np.concatenate(
        [
            np.arange((4 * r + 2 * pair) * HD, (4 * r + 2 * pair + 2) * HD)
            for pair in range(NPAIR)
            for r in range(GROUP)
        ]
    )

    in_maps = []
    for core in range(N_CORES):
        b, g = core % 2, core // 2
        hs = list(range(g * HLOC, (g + 1) * HLOC))
        xt = np.ascontiguousarray(x[b].T).astype(bfl)
        qcols = np.concatenate(
            [w_attn[:, h * HD : (h + 1) * HD] for h in hs], axis=1
        )
        kcols = np.concatenate(
            [w_attn[:, D + h * HD : D + (h + 1) * HD] for h in hs], axis=1
        )
        vcols = np.concatenate(
            [w_attn[:, 2 * D + h * HD : 2 * D + (h + 1) * HD] for h in hs],
            axis=1,
        )
        wqk = np.concatenate([qcols, kcols], axis=1).astype(bfl)
        bqv = np.concatenate([b_attn[h * HD : (h + 1) * HD] for h in hs])
        bq = bqv.reshape(2, 128).T.copy()  # [128, pair]
        bvv = np.concatenate(
            [b_attn[2 * D + h * HD : 2 * D + (h + 1) * HD] for h in hs]
        )
        bv = np.broadcast_to(bvv, (128, 256)).copy()
        wpc = np.ascontiguousarray(
            w_proj[perm][:, g * OC : (g + 1) * OC]
        ).astype(bfl)
        bpc = b_proj[g * OC : (g + 1) * OC].reshape(2, 128).T.copy()
        in_maps.append(
            dict(
                xt=xt, wqk=wqk, wv=vcols.astype(bfl), bq=bq, bv=bv,
                wp=wpc, bp=bpc, msk=msk,
            )
        )
    return in_maps


def _unshard(results):
    y = np.empty((B, S, D), np.float32)
    for core in range(N_CORES):
        b, g = core % 2, core // 2
        y[b, :, g * OC : (g + 1) * OC] = results[core]["y"].T
    return y


_NC_CACHE = {}


def kernel(x, w_attn, b_attn, w_proj, b_proj):
    from concourse.bass_utils import run_bass_kernel_spmd

    if S not in _NC_CACHE:
        _NC_CACHE[S] = _build_bass(S)
    nc = _NC_CACHE[S]
    in_maps = _shard_inputs(x, w_attn, b_attn, w_proj, b_proj)
    res = run_bass_kernel_spmd(nc, in_maps, list(range(N_CORES)))
    return _unshard(res.results)


# revision 43
# speedup vs baseline: 1.0566x; 1.0566x over previous
"""Trainium2 Bass kernel for a GPT-style causal attention block.

  y = proj( softmax_causal( (x@Wq)(x@Wk)^T / sqrt(hd) ) @ (x@Wv) )

Shapes (hardcoded): B=2, S=2048, D=1024, H=16 heads, hd=64.

Sharding over 8 NeuronCores: core = (batch b, head-group g), g selects 4
heads (2 head PAIRS). Each core:

  phase 1+2 (fused): QKV projection chains are PREFETCHED into the
    attention matmul stream one query-block ahead (the PE never drains at
    block seams, keeping its DVFS p-state at the 2.4 GHz ceiling). q,k
    produced TRANSPOSED [head_ch, S]; k gets NO bias add (a key bias
    shifts every logit of a softmax row equally - provably cancels), the
    q bias is one DVE scalar-add per block. v is natural [S, head_ch]
    padded with 64 ONES columns: the AV matmul then replicates the
    softmax denominator across psum rows 64..127 for free. Scores run as
    head PAIRS in the transposed layout [key, query] with the two K=64
    matmuls CONCURRENT in the PE via row-group tile_position; exp on ACT
    (scale folded); causal multiply only on the [128,128] diagonal
    triangle (DVE). AV(jt) is emitted after scores(jt+1) so exp latency
    never head-of-line blocks the PE. Normalize (reciprocal * av) is
    emitted mid-stream right after the diagonal masks of the NEXT query
    block, one block deferred.

  phase 3 (distributed): SIX fine-grained AllGathers, each triggered
    mid-stream the moment its aT span is normalized (pair 0 in halves,
    pair 1 per query block), so each collective's ~10-40us ncfw flight
    hides under the remaining attention stream instead of serializing at
    the end (remote_dma SBUF->SBUF peer writes were tried and are
    architecturally slow here: one descriptor per partition = ~75us per
    [128,512] chunk). The output projection consumes gathered rows per
    s-quarter: sq0 interleaved late into the last attention block, sq1-3
    in the tail where sq1/sq2 (data long arrived) cover the final
    collective's flight before sq3 needs it.

DMA order is critical-path aware: the q/k weight slices pair 0 needs and
the first quarter of xT are issued first, the rest streams under compute.
"""

import os as _os

# collectives leave mesh state behind; a stale mesh in a fresh process
# fails with "mesh desynced" unless the cores reset at runtime init
_os.environ.setdefault("NEURON_RT_RESET_CORES", "1")

import numpy as np

B = 2
S = 2048
D = 1024
H = 16
HD = 64
HLOC = 4          # heads per core
NPAIR = 2         # head pairs per core
N_CORES = 8
GROUP = 4         # cores per batch (replica group size)
IB = 512          # query block width (matmul moving dim)
OC = D // GROUP   # output-projection column shard per core (256)
SCALE = 1.0 / 8.0  # 1/sqrt(hd)


def _build_bass(s=S):
    """Build the SPMD Bass program (one NeuronCore's view)."""
    import concourse.bacc as bacc
    import concourse.mybir as mybir
    import concourse.tile as tile

    f32 = mybir.dt.float32
    f32r = mybir.dt.float32r
    bf16 = mybir.dt.bfloat16
    Alu = mybir.AluOpType
    Act = mybir.ActivationFunctionType

    n_ib = s // IB           # query blocks (4)
    n_st = s // 128          # 128-row sequence tiles (16)
    n_dt = D // 128          # contraction tiles for D (8)

    nc = bacc.Bacc(num_devices=N_CORES)

    xt = nc.declare_dram_parameter("xt", [D, s], bf16, isOutput=False)
    wqk = nc.declare_dram_parameter("wqk", [D, 512], bf16, isOutput=False)
    wv = nc.declare_dram_parameter("wv", [D, 256], bf16, isOutput=False)
    bq = nc.declare_dram_parameter("bq", [128, NPAIR], f32, isOutput=False)
    bv = nc.declare_dram_parameter("bv", [128, 256], f32, isOutput=False)
    wp = nc.declare_dram_parameter("wp", [D, OC], bf16, isOutput=False)
    bp = nc.declare_dram_parameter("bp", [128, 2], f32, isOutput=False)
    msk = nc.declare_dram_parameter("msk", [128, 128], f32, isOutput=False)
    # y TRANSPOSED [oc, s]: the projection runs as long psum chains
    y = nc.declare_dram_parameter("y", [OC, s], f32, isOutput=True)
    import os as _os
    _dbg = bool(_os.environ.get("KERNEL_DEBUG_DUMP"))
    if _dbg:
        adbg = nc.declare_dram_parameter(
            "adbg", [128, NPAIR * s], bf16, isOutput=True
        )

    with tile.TileContext(nc) as tc:
        with (
            tc.tile_pool(name="const", bufs=1) as const,
            tc.tile_pool(name="persist", bufs=1) as persist,
            tc.tile_pool(name="dram", bufs=1, space="DRAM") as dram,
        ):
            # ---- allocations (emit nothing) ----
            bq_sb = const.tile([128, NPAIR], f32)
            bv_sb = const.tile([128, 256], f32)
            bp_sb = const.tile([128, 2], f32)
            msk_sb = const.tile([128, 128], f32)
            wp_sb = const.tile([128, n_dt, OC], bf16)
            warm_sb = const.tile([1, 1], f32)
            ones_sb = const.tile([128, 256], f32)

            xt_sb = persist.tile([128, n_dt, s], bf16)
            wqk_sb = persist.tile([128, n_dt, 512], bf16)
            wv_sb = persist.tile([128, n_dt, 256], bf16)
            # bf16 score/AV operands: halves LDWEIGHTS traffic (the AV
            # lhsT is reloaded per key tile) and dodges the f32r
            # small-moving-dim matmul penalty on diagonal tiles
            qT_sb = persist.tile([128, NPAIR, s], bf16)   # [pair_ch, pair, s]
            kT_sb = persist.tile([128, NPAIR, s], bf16)
            # [:, st, h, 0:64] = v channels, [:, st, h, 64:128] = 1.0
            v_sb = persist.tile([128, n_st, HLOC, 128], bf16)
            aT_sb = persist.tile([128, NPAIR, s], bf16)
            # gathered rows: [:, pair, rank, :] = rank's pair rows
            agf_sb = persist.tile([128, NPAIR, GROUP, s], bf16)

            # fine-grained collective spans: (pair, col_lo, col_hi),
            # triggered right after normalize of (key_pair, key_ib)
            AGS = {
                (0, 0): (0, 0, IB),              # pair0 per block: front-
                (0, 1): (0, IB, s // 2),         # load the CC pipeline
                (0, 2): (0, s // 2, 3 * IB),
                (0, 3): (0, 3 * IB, s),
                (1, 1): (1, 0, s // 2),          # pair1 first half
                (1, 2): (1, s // 2, 3 * IB),     # pair1 third quarter
                (1, 3): (1, 3 * IB, s),          # pair1 last block (small:
            }                                    # its flight is the tail)
            ag_in = {
                k: dram.tile([128, hi - lo], bf16, name=f"agi{k[0]}{k[1]}")
                for k, (p, lo, hi) in AGS.items()
            }
            ag_out = {
                k: dram.tile(
                    [GROUP * 128, hi - lo], bf16, name=f"ago{k[0]}{k[1]}"
                )
                for k, (p, lo, hi) in AGS.items()
            }

            # ---- DMA emission, critical-path order ----
            # 1. exactly what pair-0 block-0 needs: q/k pair-0 weight
            #    slices + first xT quarter.
            wqk_r = wqk.rearrange("(t p) c -> p t c", p=128)
            xt_r = xt.rearrange("(t p) ss -> p t ss", p=128)
            nc.sync.dma_start(out=wqk_sb[:, :, 0:128], in_=wqk_r[:, :, 0:128])
            # first xT quarter in dt-halves: the qk chain's dt-0..3
            # matmuls start before the 4..7 tiles land
            nc.sync.dma_start(
                out=xt_sb[:, 0:4, 0 : s // 4], in_=xt_r[:, 0:4, 0 : s // 4]
            )
            nc.sync.dma_start(
                out=wqk_sb[:, :, 256:384], in_=wqk_r[:, :, 256:384]
            )
            nc.sync.dma_start(
                out=xt_sb[:, 4:8, 0 : s // 4], in_=xt_r[:, 4:8, 0 : s // 4]
            )
            nc.sync.dma_start(out=bq_sb, in_=bq[:, :])
            nc.sync.dma_start(
                out=wv_sb, in_=wv.rearrange("(t p) c -> p t c", p=128)
            )
            nc.sync.dma_start(out=msk_sb, in_=msk[:, :])
            nc.sync.dma_start(out=bv_sb, in_=bv[:, :])
            # dummy exp: pulls the ACT exp table load off the critical path
            nc.scalar.activation(
                out=warm_sb, in_=bq_sb[0:1, 0:1], func=Act.Exp, scale=0.0
            )
            nc.vector.memset(ones_sb, 1.0)
            # 2. rest streams under compute
            nc.sync.dma_start(
                out=wqk_sb[:, :, 128:256], in_=wqk_r[:, :, 128:256]
            )
            nc.sync.dma_start(
                out=wqk_sb[:, :, 384:512], in_=wqk_r[:, :, 384:512]
            )
            for q in range(1, 4):
                nc.sync.dma_start(
                    out=xt_sb[:, :, q * s // 4 : (q + 1) * s // 4],
                    in_=xt_r[:, :, q * s // 4 : (q + 1) * s // 4],
                )
            nc.sync.dma_start(out=bp_sb, in_=bp[:, :])
            nc.sync.dma_start(
                out=wp_sb, in_=wp.rearrange("(t p) c -> p t c", p=128)
            )

            with (
                tc.tile_pool(name="ps_s", bufs=2, space="PSUM") as ps_s,
                tc.tile_pool(name="ps_av", bufs=2, space="PSUM") as ps_av,
                tc.tile_pool(name="pt", bufs=5) as ptpool,
                tc.tile_pool(name="small", bufs=6) as small,
                tc.tile_pool(name="yout", bufs=3) as yout,
            ):
                # ---------- chain emitters (PE work units) ----------
                def v_chain(st):
                    # v natural: lhsT = xT tile [d, s-tile], rhs = Wv
                    psv = ps_s.tile([128, 256], f32, name="psv", tag="pss")
                    for dt in range(n_dt):
                        nc.tensor.matmul(
                            psv,
                            lhsT=(xt_sb[:, dt, st * 128 : (st + 1) * 128]),
                            rhs=(wv_sb[:, dt, :]),
                            start=(dt == 0),
                            stop=(dt == n_dt - 1),
                        )
                    nc.vector.tensor_tensor(
                        out=v_sb[:, st, :, 0:64],
                        in0=psv.rearrange("p (h e) -> p h e", h=HLOC),
                        in1=bv_sb.rearrange("p (h e) -> p h e", h=HLOC),
                        op=Alu.add,
                    )
                    # ones columns: AV replicates the softmax denominator
                    # across psum rows 64..127
                    nc.vector.tensor_copy(
                        out=v_sb[:, st, :, 64:128],
                        in_=ones_sb.rearrange("p (h e) -> p h e", h=HLOC),
                    )

                def qk_chain(t, sb):
                    # qT/kT: lhsT = W tile [d,c], rhs = xT [d, s-block]
                    # c-tile t: 0,1 = q pair0/1; 2,3 = k pair0/1
                    ps = ps_s.tile([128, IB], f32, name="ps", tag="pss")
                    for dt in range(n_dt):
                        nc.tensor.matmul(
                            ps,
                            lhsT=(wqk_sb[:, dt, t * 128 : (t + 1) * 128]),
                            rhs=(xt_sb[:, dt, sb * IB : (sb + 1) * IB]),
                            start=(dt == 0),
                            stop=(dt == n_dt - 1),
                        )
                    if t < 2:  # q: bias add (k bias cancels in softmax)
                        nc.vector.tensor_scalar_add(
                            out=qT_sb[:, t, sb * IB : (sb + 1) * IB],
                            in0=ps,
                            scalar1=bq_sb[:, t : t + 1],
                        )
                    else:
                        nc.vector.tensor_copy(
                            out=kT_sb[:, t - 2, sb * IB : (sb + 1) * IB],
                            in_=ps,
                        )

                def proj_chain(sq, ocb):
                    """yT[ocb rows, s-quarter sq] = sum_t wp_t^T @ a_t."""
                    lo, hi = sq * s // 4, (sq + 1) * s // 4
                    yt = ps_s.tile([128, s // 4], f32, name="yt", tag="pss")
                    t = 0
                    for pair in range(NPAIR):
                        for r in range(GROUP):
                            nc.tensor.matmul(
                                yt,
                                lhsT=(wp_sb[:, t, ocb * 128 : (ocb + 1) * 128]),
                                rhs=agf_sb[:, pair, r, lo:hi],
                                start=(t == 0),
                                stop=(t == 2 * GROUP - 1),
                            )
                            t += 1
                    ysb = yout.tile([128, s // 4], f32, name="ysb")
                    nc.vector.tensor_scalar_add(
                        out=ysb, in0=yt, scalar1=bp_sb[:, ocb : ocb + 1]
                    )
                    nc.sync.dma_start(
                        out=y[ocb * 128 : (ocb + 1) * 128, lo:hi], in_=ysb
                    )

                def proj_emit(sq):
                    proj_chain(sq, 0)
                    proj_chain(sq, 1)

                # ---------- softmax-side emitters ----------
                def normalize_emit(pair, ib, avs):
                    """aT[:, ib block] = av[0:64] * (1 / av[64:128])."""
                    for hh in range(2):
                        zsb = small.tile([64, IB], f32, name="zsb")
                        nc.vector.tensor_copy(out=zsb, in_=avs[hh][64:128, :])
                        recr = small.tile([64, IB], f32, name="recr")
                        nc.vector.reciprocal_approx_fast(out=recr, in_=zsb)
                        nc.vector.tensor_tensor(
                            out=aT_sb[
                                hh * 64 : (hh + 1) * 64,
                                pair,
                                ib * IB : (ib + 1) * IB,
                            ],
                            in0=recr,
                            in1=avs[hh][0:64, :],
                            op=Alu.mult,
                        )

                def send_emit(pair, ib):
                    """Stage + AllGather + reload the collective span
                    keyed by (pair, ib), if any."""
                    key = (pair, ib)
                    if key not in AGS:
                        return
                    p, lo, hi = AGS[key]
                    gin, gout = ag_in[key], ag_out[key]
                    nc.sync.dma_start(out=gin, in_=aT_sb[:, p, lo:hi])
                    nc.gpsimd.collective_compute(
                        "AllGather",
                        Alu.bypass,
                        replica_groups=[[0, 2, 4, 6], [1, 3, 5, 7]],
                        ins=[gin[:, :]],
                        outs=[gout[:, :]],
                    )
                    nc.sync.dma_start(
                        out=agf_sb[:, p, :, lo:hi],
                        in_=gout.rearrange("(r p) ss -> p r ss", p=128),
                    )

                # ---------- the fused attention stream ----------
                def scores_av_emit(pair, ib, fillers, pending):
                    """Scores + exp + causal mask + AV for query block ib.
                    `fillers` are chain thunks interleaved into the jt
                    stream; `pending` (avs of the previous block) is
                    normalized + shipped right after the diagonal masks.
                    Returns the two [128,IB] psum accumulators (rows
                    64..127 = replicated softmax denominator)."""
                    njt = n_ib * (ib + 1)  # key tiles needed (j <= i)
                    avs = [
                        ps_av.tile([128, IB], f32, name=f"av{hh}", tag=f"av{hh}")
                        for hh in range(2)
                    ]
                    # diagonal key tiles first: their mask multiply then
                    # overlaps the long non-diagonal score/AV stream
                    jt_order = list(range(n_ib * ib, njt)) + list(
                        range(n_ib * ib)
                    )
                    fill = list(fillers)
                    # spread fillers over the post-diagonal jt slots
                    slots = {}
                    nseg = len(fill) + 1
                    for m in range(len(fill)):
                        pos = 4 + (m + 1) * max(njt - 4, 1) // nseg
                        slots.setdefault(min(pos, njt - 1), []).append(fill[m])

                    def av_emit(pt, jt, lo, jseq):
                        for hh in range(2):
                            nc.tensor.matmul(
                                avs[hh][:, lo:IB],
                                lhsT=(v_sb[:, jt, pair * 2 + hh, :]),
                                rhs=(pt[:, hh, lo:IB]),
                                start=(jseq == 0),
                                stop=(jseq == njt - 1),
                            )

                    prev_av = None
                    for jseq, jt in enumerate(jt_order):
                        k = jt - n_ib * ib  # >= 0: diagonal tile index
                        lo = 128 * k if k > 0 else 0
                        pss = ps_s.tile(
                            [128, 2 * IB], f32, name="pss", tag="pss"
                        )
                        for hh in range(2):
                            off = hh * 64
                            nc.tensor.matmul(
                                pss[:, hh * IB + lo : (hh + 1) * IB],
                                lhsT=(kT_sb[
                                        off : off + 64,
                                        pair,
                                        jt * 128 : (jt + 1) * 128,
                                    ]
                                ),
                                rhs=(qT_sb[
                                        off : off + 64,
                                        pair,
                                        ib * IB + lo : (ib + 1) * IB,
                                    ]
                                ),
                                start=True,
                                stop=True,
                                tile_position=(off, 0),
                            )
                        if prev_av is not None:
                            av_emit(*prev_av)
                        pt = ptpool.tile([128, 2, IB], bf16, name="pt")
                        nc.scalar.activation(
                            out=pt[:, :, lo:IB],
                            in_=pss.rearrange("p (h q) -> p h q", h=2)[
                                :, :, lo:IB
                            ],
                            func=Act.Exp,
                            scale=SCALE,
                        )
                        for hh in range(2):
                            if k >= 0:  # causal triangle on diagonal block
                                nc.vector.tensor_tensor(
                                    out=pt[:, hh, lo : lo + 128],
                                    in0=pt[:, hh, lo : lo + 128],
                                    in1=msk_sb,
                                    op=Alu.mult,
                                )
                        prev_av = (pt, jt, lo, jseq)
                        # after the diagonals: previous block's normalize
                        # + collective ship, then any prefetch fillers
                        if jseq == min(3, njt - 1) and pending is not None:
                            p_pair, p_ib, p_avs = pending
                            normalize_emit(p_pair, p_ib, p_avs)
                            send_emit(p_pair, p_ib)
                            pending = None
                        for th in slots.get(jseq, ()):
                            th()
                    av_emit(*prev_av)
                    return avs

                # ---------- main schedule ----------
                # pipeline fill: q/k chains FIRST (their weights are the
                # first DMAs to land; v chains need wv which arrives
                # later, and the PE queue is in-order)
                qk_chain(0, 0)
                qk_chain(2, 0)
                for st in range(4):
                    v_chain(st)

                seq = [(p, ib) for p in range(NPAIR) for ib in range(n_ib)]
                pending = None
                for i, (pair, ib) in enumerate(seq):
                    nxt = seq[i + 1] if i + 1 < len(seq) else None
                    fillers = []
                    if pair == 0 and ib < n_ib - 1:
                        # next block's v tiles
                        fillers += [
                            (lambda st=st: v_chain(st))
                            for st in range(4 * (ib + 1), 4 * (ib + 2))
                        ]
                    if nxt is not None:
                        # prefetch the next stream's q/k blocks
                        fillers += [
                            lambda t=nxt[0], sb=nxt[1]: qk_chain(t, sb),
                            lambda t=2 + nxt[0], sb=nxt[1]: qk_chain(t, sb),
                        ]
                    if (pair, ib) == (1, 3):
                        # sq0+sq1 in the last block's stream (both need
                        # only pair0 quarters + pair1 first half, all
                        # gathered long ago); single chains per slot so
                        # both psum slots are never held at once
                        fillers += [
                            lambda: proj_chain(0, 0),
                            lambda: proj_chain(0, 1),
                            lambda: proj_chain(1, 0),
                            lambda: proj_chain(1, 1),
                        ]
                    avs = scores_av_emit(pair, ib, fillers, pending)
                    pending = (pair, ib, avs)
                # tail: last chunk's normalize + collective; quarter sq2
                # (data long arrived) covers its flight before sq3
                p_pair, p_ib, p_avs = pending
                normalize_emit(p_pair, p_ib, p_avs)
                send_emit(p_pair, p_ib)
                proj_emit(2)
                proj_emit(3)
                if _dbg:
                    nc.sync.dma_start(
                        out=adbg[:, :], in_=aT_sb.rearrange("p a b -> p (a b)")
                    )

    nc.compile()
    return nc


def _shard_inputs(x, w_attn, b_attn, w_proj, b_proj, s=S):
    """Host-side sharding: build the per-core input maps."""
    import ml_dtypes

    bfl = ml_dtypes.bfloat16
    x = np.asarray(x, dtype=np.float32)
    w_attn = np.asarray(w_attn, dtype=np.float32)
    b_attn = np.asarray(b_attn, dtype=np.float32)
    w_proj = np.asarray(w_proj, dtype=np.float32)
    b_proj = np.asarray(b_proj, dtype=np.float32)

    # causal triangle tile: msk[j, i] = 1.0 if i >= j
    msk = (np.arange(128)[None, :] >= np.arange(128)[:, None]).astype(
        np.float32
    )

    # w_proj rows in gather-rank order: t = (pair, rank): rows of
    # head-group `rank`'s heads (2*pair, 2*pair+1)
    perm = np.concatenate(
        [
            np.arange((4 * r + 2 * pair) * HD, (4 * r + 2 * pair + 2) * HD)
            for pair in range(NPAIR)
            for r in range(GROUP)
        ]
    )

    in_maps = []
    for core in range(N_CORES):
        b, g = core % 2, core // 2
        hs = list(range(g * HLOC, (g + 1) * HLOC))
        xt = np.ascontiguousarray(x[b].T).astype(bfl)
        qcols = np.concatenate(
            [w_attn[:, h * HD : (h + 1) * HD] for h in hs], axis=1
        )
        kcols = np.concatenate(
            [w_attn[:, D + h * HD : D + (h + 1) * HD] for h in hs], axis=1
        )
        vcols = np.concatenate(
            [w_attn[:, 2 * D + h * HD : 2 * D + (h + 1) * HD] for h in hs],
            axis=1,
        )
        wqk = np.concatenate([qcols, kcols], axis=1).astype(bfl)
        bqv = np.concatenate([b_attn[h * HD : (h + 1) * HD] for h in hs])
        bq = bqv.reshape(2, 128).T.copy()  # [128, pair]
        bvv = np.concatenate(
            [b_attn[2 * D + h * HD : 2 * D + (h + 1) * HD] for h in hs]
        )
        bv = np.broadcast_to(bvv, (128, 256)).copy()
        wpc = np.ascontiguousarray(
            w_proj[perm][:, g * OC : (g + 1) * OC]
        ).astype(bfl)
        bpc = b_proj[g * OC : (g + 1) * OC].reshape(2, 128).T.copy()
        in_maps.append(
            dict(
                xt=xt, wqk=wqk, wv=vcols.astype(bfl), bq=bq, bv=bv,
                wp=wpc, bp=bpc, msk=msk,
            )
        )
    return in_maps


def _unshard(results):
    y = np.empty((B, S, D), np.float32)
    for core in range(N_CORES):
        b, g = core % 2, core // 2
        y[b, :, g * OC : (g + 1) * OC] = results[core]["y"].T
    return y


_NC_CACHE = {}


def kernel(x, w_attn, b_attn, w_proj, b_proj):
    from concourse.bass_utils import run_bass_kernel_spmd

    if S not in _NC_CACHE:
        _NC_CACHE[S] = _build_bass(S)
    nc = _NC_CACHE[S]
    in_maps = _shard_inputs(x, w_attn, b_attn, w_proj, b_proj)
    # a predecessor process that crashed mid-collective can leave the
    # mesh in a state that fails one execution with "mesh desynced";
    # the state clears on the next attempt, so retry rather than die
    last = None
    for _ in range(3):
        try:
            res = run_bass_kernel_spmd(nc, in_maps, list(range(N_CORES)))
            return _unshard(res.results)
        except Exception as e:  # transient runtime desync/unavailable
            last = e
    raise last
